# revision 1
# baseline (speedup 1.0000x reference)
"""Trainium2 Bass kernel for a single causal attention head (bf16 compute).

Reference (per batch element b):
    q = x[b] @ Wq; k = x[b] @ Wk; v = x[b] @ Wv          # [T, HD]
    S = q @ k.T;  S = where(tril, S, -inf) / sqrt(C)
    out[b] = softmax(S, -1) @ v                           # [T, HD]

Sharding: pure data parallel -- core i computes batch element i
(B == 8 == n_cores). No collectives.

Device algorithm (per core), built to avoid large transposes:
  * host pre-transposes x[b] -> xT [C, T] so the contraction dim (C) lies
    on SBUF partitions with unit-stride DMA; weights are pre-interleaved as
    [Wq_i | Wk_i] so one matmul computes qT and kT stacked.
  * scores are computed TRANSPOSED: S_T[s, t] = kT_slice.T @ qT, so that
    exp(S_T) (ScalarE, 1/sqrt(C) folded into the activation scale) is
    directly the moving operand of the second matmul, whose stationary is v
    in natural layout with a ones column appended: one matmul yields both
    out_unnorm.T and the softmax row-sums. Causal masking = one constant
    [128,512] triangular tile multiplied on VectorE (diagonal blocks only);
    fully masked blocks are never computed, partially masked blocks are
    column-trimmed.
  * v is projected directly into natural layout using xT slices as the
    matmul stationary; normalization transposes small [65,128] tiles on
    TensorE and divides by the row-sum reciprocal on VectorE.
  * schedule: per chunk, projection then all scores+exp (ScalarE starts
    early and stays fed); all attv accumulation runs as a dense TensorE
    stream afterwards; a PE warmup burst covers the initial DMA window.
"""

import numpy as np

B, T, C, HD = 8, 2048, 1024, 64
NCORES = 8
CHUNK = 512
NJ = T // CHUNK
NCT = C // 128
NST = T // 128
SCALE = 1.0 / np.sqrt(np.float32(C))
XP = 2                      # t-pieces per c-tile for input DMA
XPW = T // XP

MODE = "bf16"
WARMUP_MM = 30
WARMUP_N = 32


def build_bass(mode=MODE, reps=1):
    import concourse.bacc as bacc
    import concourse.tile as tile
    import concourse.mybir as mybir
    from concourse.masks import make_identity

    f32 = mybir.dt.float32
    if mode == "bf16":
        st_dt = mybir.dt.bfloat16
    elif mode == "f32r":
        st_dt = mybir.dt.float32r
    else:
        st_dt = f32

    EXP = mybir.ActivationFunctionType.Exp
    GE = mybir.AluOpType.is_ge
    DIV = mybir.AluOpType.divide

    nc = bacc.Bacc("TRN2", target_bir_lowering=False, debug=False,
                   num_devices=NCORES)
    xt = nc.dram_tensor("xt", [NCT, XP, 128, XPW], st_dt,
                        kind="ExternalInput")
    w = nc.dram_tensor("w", [128, NCT * 192], st_dt, kind="ExternalInput")
    out = nc.dram_tensor("out", [T, HD], f32, kind="ExternalOutput")

    with tile.TileContext(nc) as tc:
        with (
            tc.tile_pool(name="consts", bufs=1) as consts,
            tc.tile_pool(name="xin", bufs=NCT * XP) as xin,
            tc.tile_pool(name="proj", bufs=1) as proj,
            tc.tile_pool(name="es", bufs=41) as es_pool,
            tc.tile_pool(name="small", bufs=4) as small,
            tc.tile_pool(name="psA", bufs=3, space="PSUM") as psA,
            tc.tile_pool(name="psQK", bufs=1, space="PSUM") as psQK,
            tc.tile_pool(name="psO", bufs=2, space="PSUM") as psO,
            tc.tile_pool(name="psT", bufs=2, space="PSUM") as psT,
        ):
            # PE warmup source: zeroed by DVE so PE can start ~immediately,
            # keeping the HAM clock-gate warm while input DMAs stream in.
            warm_src = consts.tile([128, WARMUP_N], f32, tag="warm")
            nc.vector.memset(warm_src[:], 0.0)
            warm_ps = psT.tile([128, WARMUP_N], f32, tag="tp")
            for _w in range(WARMUP_MM):
                nc.tensor.matmul(warm_ps[0:WARMUP_N, :], warm_src[:],
                                 warm_src[:], start=True, stop=True)

            # weights first: first projection needs them
            w_sb = consts.tile([128, NCT * 192], st_dt, tag="w")
            nc.sync.dma_start(w_sb[:], w[:, :])
            wqk_sb = w_sb[:, 0:NCT * 128]
            wv_sb = w_sb[:, NCT * 128:NCT * 192]

            ident_f = consts.tile([128, 128], f32, tag="ident_f")
            make_identity(nc, ident_f[:])

            # causal mask M[s, y] = 1 if y >= s else 0  (shared by all
            # diagonal blocks; diagonal block r uses M[:, 0:512-128r])
            cmask = consts.tile([128, CHUNK], st_dt, tag="cmask")
            nc.gpsimd.memset(cmask[:], 1.0)
            nc.gpsimd.affine_select(
                out=cmask[:], in_=cmask[:], compare_op=GE, fill=0.0,
                base=0, channel_multiplier=-1, pattern=[[1, CHUNK]],
            )

            for _rep in range(reps):
                emit_body(nc, tc, st_dt, f32, EXP, DIV, ident_f, cmask,
                          wqk_sb, wv_sb, proj, xin, es_pool, small,
                          psA, psQK, psO, psT, xt, out)

    nc.compile()
    return nc


def emit_body(nc, tc, st_dt, f32, EXP, DIV, ident_f, cmask, wqk_sb, wv_sb,
              proj, xin, es_pool, small, psA, psQK, psO, psT, xt, out):
    q_sb = proj.tile([64, T], st_dt, tag="q")
    k_sb = proj.tile([64, T], st_dt, tag="k")
    v65 = proj.tile([128, NST * 65], st_dt, tag="v65")
    for st in range(NST):
        nc.gpsimd.memset(v65[:, st * 65 + 64: st * 65 + 65], 1.0)

    xts = {}
    for p in range(XP):
        for i in range(NCT):
            xtile = xin.tile([128, XPW], st_dt, tag="x")
            nc.sync.dma_start(xtile[:], xt[i, p, :, :])
            xts[i, p] = xtile

    def xpiece(i, j):
        p = j // (NJ // XP)
        sub = j % (NJ // XP)
        return xts[i, p][:, sub * CHUNK:(sub + 1) * CHUNK]

    ess = {}

    for j in range(NJ):
        # ---- q,k projections (stacked stationary [Wq_i | Wk_i]) ----
        ps_qk = psQK.tile([128, CHUNK], f32, tag="qk")
        for i in range(NCT):
            nc.tensor.matmul(
                ps_qk[:],
                wqk_sb[:, i * 128:(i + 1) * 128],
                xpiece(i, j),
                start=(i == 0), stop=(i == NCT - 1),
            )
        tsl = slice(j * CHUNK, (j + 1) * CHUNK)
        nc.vector.tensor_copy(q_sb[:, tsl], ps_qk[0:64, :])
        nc.vector.tensor_copy(k_sb[:, tsl], ps_qk[64:128, :])

        # ---- v projection, direct natural layout (xT slice stationary) ----
        for r in range(4):
            st = 4 * j + r
            ps_v = psT.tile([128, HD], f32, tag="tp")
            for i in range(NCT):
                nc.tensor.matmul(
                    ps_v[:],
                    xpiece(i, j)[:, r * 128:(r + 1) * 128],
                    wv_sb[:, i * HD:(i + 1) * HD],
                    start=(i == 0), stop=(i == NCT - 1),
                )
            nc.vector.tensor_copy(v65[:, st * 65: st * 65 + HD], ps_v[:, :])

        # ---- scores + exp + mask for t-chunk j (attv deferred) ----
        nst = 4 * (j + 1)
        for st in range(nst):
            r = st - 4 * j           # >=0 on diagonal tiles
            off = 128 * r if r > 0 else 0   # causal column trim
            n = CHUNK - off
            ps_s = psA.tile([128, CHUNK], f32, tag="mm")
            nc.tensor.matmul(
                ps_s[:, off:CHUNK],
                k_sb[:, st * 128:(st + 1) * 128],
                q_sb[:, j * CHUNK + off:(j + 1) * CHUNK],
                start=True, stop=True,
            )
            es = es_pool.tile([128, CHUNK], st_dt, tag="es")
            nc.scalar.activation(es[:, off:CHUNK], ps_s[:, off:CHUNK], EXP,
                                 scale=float(SCALE))
            if r >= 0:
                # zero the sub-diagonal half: es[s, y] *= (y >= s)
                nc.vector.tensor_mul(es[:, off:CHUNK], es[:, off:CHUNK],
                                     cmask[:, 0:n])
            ess[j, st] = (es, off)

    # ---- attv accumulation + normalize, chunk by chunk ----
    for j in range(NJ):
        ps_oT = psO.tile([128, CHUNK], f32, tag="oT")
        nst = 4 * (j + 1)
        for st in range(nst):
            es, off = ess[j, st]
            nc.tensor.matmul(
                ps_oT[0:65, off:CHUNK],
                v65[:, st * 65:(st + 1) * 65],
                es[:, off:CHUNK],
                start=(st == 0), stop=(st == nst - 1),
            )
        ob = small.tile([128, 4 * HD], f32, tag="ob")
        for kk in range(4):
            oT_piece = small.tile([65, 128], f32, tag="oT_piece")
            nc.vector.tensor_copy(oT_piece[:, :],
                                  ps_oT[0:65, kk * 128:(kk + 1) * 128])
            ps_o = psT.tile([128, 65], f32, tag="tp")
            nc.tensor.transpose(ps_o[:], oT_piece[:, :], ident_f[0:65, 0:65])
            rec = small.tile([128, 1], f32, tag="rec")
            nc.vector.reciprocal(rec[:], ps_o[:, 64:65])
            nc.vector.tensor_scalar_mul(ob[:, kk * HD:(kk + 1) * HD],
                                        ps_o[:, 0:HD], rec[:])
            # ship each 128-row block as soon as it is normalized so the
            # kernel tail is not one serial copy+transfer chain
            tb = 4 * j + kk
            nc.sync.dma_start(out[tb * 128:(tb + 1) * 128, :],
                              ob[:, kk * HD:(kk + 1) * HD])


def prep_inputs(x, Wq, Wk, Wv, mode=MODE):
    if mode == "bf16":
        import ml_dtypes
        cast = lambda a: np.ascontiguousarray(a).astype(ml_dtypes.bfloat16)
    else:
        cast = lambda a: np.ascontiguousarray(a, dtype=np.float32)

    wq_r = Wq.reshape(NCT, 128, HD)
    wk_r = Wk.reshape(NCT, 128, HD)
    wqk = np.concatenate([wq_r, wk_r], axis=2)
    wv = Wv.reshape(NCT, 128, HD)
    wqk = wqk.transpose(1, 0, 2).reshape(128, NCT * 128)
    wvt = wv.transpose(1, 0, 2).reshape(128, NCT * HD)
    wfull = cast(np.concatenate([wqk, wvt], axis=1))

    in_maps = []
    for b in range(NCORES):
        xtb = x[b].T
        xtb = xtb.reshape(NCT, 128, XP, XPW).transpose(0, 2, 1, 3)
        in_maps.append({"xt": cast(xtb), "w": wfull})
    return in_maps


_NC_CACHE = {}


def kernel(x, Wq, Wk, Wv):
    from concourse.bass_utils import run_bass_kernel_spmd

    if MODE not in _NC_CACHE:
        _NC_CACHE[MODE] = build_bass(MODE)
    nc = _NC_CACHE[MODE]
    in_maps = prep_inputs(np.asarray(x), np.asarray(Wq), np.asarray(Wk),
                          np.asarray(Wv), MODE)
    res = run_bass_kernel_spmd(nc, in_maps, core_ids=list(range(NCORES)))
    return np.stack([res.results[b]["out"] for b in range(NCORES)], axis=0)

